# revision 48
# baseline (speedup 1.0000x reference)
"""Disentangled MHA (DeBERTa-style) Trainium2 Bass kernel.

Sharding: 16 heads across 8 cores (2 heads/core), batch kept local.
Per core: project q/k/v with a 128-column weight slice, build the
relative-position score bands, skew-gather them via a DRAM round trip,
softmax (transposed orientation, unnormalized-exp + fused Z column),
and PV matmul. Host concatenates the per-core 128-feature outputs.

Key structure: host pre-chunks inputs so every load DMA is contiguous
per partition (x loads gated behind relT so the critical path gets HBM
bandwidth first); per-batch interleaved emission (proj -> bands ->
reads -> scores -> PV as separate pipeline steps, reads issued with
their batch's band writes and QK/PV trailing ~1.5 batches); merged
whole-(b,h) skew reads (one transpose DMA on the SP HWDGE ring + one
accumulating SWDGE DMA -- note an SWDGE accum after an ACT-ring
transpose races on HW, and GpSimd cannot touch PSUM); DMA spread over
the three issuing queues; single 8-bank PSUM pool
(proj/band 3 + qk 2 + pv 2 + transposes 1).

B=4, S=512, DIM=1024, H=16, HD=64, MAX_REL=512.
"""

import numpy as np

import concourse.bass as bass
import concourse.bacc as bacc
import concourse.mybir as mybir
import concourse.tile as tile
from concourse.bass_utils import run_bass_kernel_spmd
from concourse.masks import make_identity

B, S, DIM, H, HD = 4, 512, 1024, 16, 64
T = B * S                      # 2048 tokens
R = 1024                       # 2 * att_span rel rows
HC = 2                         # heads per core
NCORES = 8
KC = DIM // 128                # contraction chunks
SCALE = float((HD * 3) ** (-0.5))
BAND = 640                     # skew band width (needs >= 512 + 127)

F32 = mybir.dt.float32
F16 = mybir.dt.float16
AF = mybir.ActivationFunctionType
ALU = mybir.AluOpType


def build_nc():
    nc = bacc.Bacc("TRN2", target_bir_lowering=False, debug=False)

    # host pre-chunked layouts: row p holds chunk-contiguous data
    xT_d = nc.dram_tensor("xT", [128, KC * T], F16, kind="ExternalInput")
    relT_d = nc.dram_tensor("relT", [128, KC * R], F16, kind="ExternalInput")
    W_d = {
        n: nc.dram_tensor(f"W{n}", [128, KC * 128], F16, kind="ExternalInput")
        for n in "qkv"
    }
    b_d = {
        n: nc.dram_tensor(f"b{n}", [128, 1], F32, kind="ExternalInput")
        for n in "qkv"
    }
    out_d = nc.dram_tensor("out", [T, 128], F32, kind="ExternalOutput")

    with tile.TileContext(nc) as tc:
        _body(nc, tc, xT_d.ap(), relT_d.ap(),
              {n: W_d[n].ap() for n in "qkv"},
              {n: b_d[n].ap() for n in "qkv"},
              out_d.ap())
    nc.compile()
    return nc


def _body(nc, tc, xT, relT, W, bvec, out_d):
    from contextlib import ExitStack
    ctx = ExitStack()
    with ctx:
        singles = ctx.enter_context(tc.tile_pool(name="singles", bufs=1))

        # ---- Input loads: all contiguous, spread over the three queues.
        relT_sb = singles.tile([128, KC * R], F16, name="relT_sb")
        W_sb = {n: singles.tile([128, KC * 128], F16, name=f"W{n}_sb")
                for n in "qkv"}
        xT_sb = singles.tile([128, KC * T], F16, name="xT_sb")
        b_t = {n: singles.tile([128, 1], F32, name=f"b{n}") for n in "qkv"}

        HKR = KC * R // 2
        QKR = KC * R // 4
        nc.sync.dma_start(out=W_sb["k"], in_=W["k"])
        nc.scalar.dma_start(out=W_sb["q"], in_=W["q"])
        # relT in quarters, alternating rings: chunk 0 lands first so the
        # posk projection chain starts streaming via subtile deps
        nc.sync.dma_start(out=relT_sb[:, 0:QKR], in_=relT[:, 0:QKR])
        nc.scalar.dma_start(out=relT_sb[:, QKR:HKR], in_=relT[:, QKR:HKR])
        nc.sync.dma_start(out=relT_sb[:, HKR:3 * QKR],
                          in_=relT[:, HKR:3 * QKR])
        nc.scalar.dma_start(out=relT_sb[:, 3 * QKR:], in_=relT[:, 3 * QKR:])
        nc.gpsimd.dma_start(out=W_sb["v"], in_=W["v"])
        BK = KC * S  # per-batch chunk-contiguous span
        # all xT loads gated behind relT: tiny copies reading relT_sb force
        # the x SDMA transfers to start only after the critical relT load
        # drains, so relT/W don't fair-share HBM bandwidth with 4MB of x.
        # xb0 releases at half-relT, xb1-3 at the tail (subtile deps).
        nc.vector.tensor_copy(xT_sb[0:1, 0:1], relT_sb[0:1, HKR - 1:HKR])
        for b in range(1, B):
            nc.vector.tensor_copy(xT_sb[0:1, b * BK:b * BK + 1],
                                  relT_sb[0:1, KC * R - 1:KC * R])
        xq = [nc.sync, nc.scalar, nc.gpsimd, nc.gpsimd]
        for b in range(B):
            xq[b].dma_start(out=xT_sb[:, b * BK:(b + 1) * BK],
                            in_=xT[:, b * BK:(b + 1) * BK])
        for n in "qkv":
            nc.gpsimd.dma_start(out=b_t[n], in_=bvec[n])

        def x_sl(kc, b):
            return xT_sb[:, (b * KC + kc) * S:(b * KC + kc) * S + S]

        def rel_sl(kc, nt):
            return relT_sb[:, kc * R + nt * 512:kc * R + nt * 512 + 512]

        W_t = {n: [W_sb[n][:, i * 128:(i + 1) * 128] for i in range(KC)]
               for n in "qkv"}

        ident = singles.tile([128, 128], F16, name="ident")
        make_identity(nc, ident)

        exp_bias = singles.tile([128, 1], F32, name="exp_bias")
        nc.gpsimd.memset(exp_bias, -4.0)

        # ---- persistent SBUF for projections ----
        q2T = singles.tile([128, T], F16, name="q2T")
        k2T = singles.tile([128, T], F16, name="k2T")
        v2T = singles.tile([128, T], F16, name="v2T")
        posk = singles.tile([128, R], F16, name="posk")
        posq = singles.tile([128, R], F16, name="posq")
        vtok = [singles.tile([128, 130], F16, name=f"vtok{t}")
                for t in range(T // 128)]

        # ---- single PSUM pool: ps512(3, shared with f16 transposes) +
        # ps_qk(3) + ps_pv(2) = 8 banks
        ps = ctx.enter_context(tc.tile_pool(name="ps", space="PSUM", bufs=1))

        sb_band = ctx.enter_context(tc.tile_pool(name="sb_band", bufs=1))
        sb_work = ctx.enter_context(tc.tile_pool(name="sb_work", bufs=1))
        band_dram = ctx.enter_context(
            tc.tile_pool(name="bands", space="DRAM", bufs=1))

        # ---- helpers ----
        def proj_tile(out_sb, rhs_sl, wn, nt):
            psx = ps.tile([128, 512], F32, name="ps_proj", tag="ps512", bufs=3)
            for kc in range(KC):
                nc.tensor.matmul(
                    out=psx, lhsT=W_t[wn][kc], rhs=rhs_sl(kc, nt),
                    start=(kc == 0), stop=(kc == KC - 1),
                )
            nc.scalar.activation(
                out=out_sb[:, nt * 512:(nt + 1) * 512], in_=psx,
                func=AF.Identity, bias=b_t[wn], scale=1.0,
            )

        cast_flip = [0, 0]

        def band_cast(dst, src):
            # 2:1 DVE:ACT for the 512-wide band psum evacuation
            if cast_flip[0] % 3 != 2:
                nc.vector.tensor_copy(dst, src)
            else:
                nc.scalar.copy(dst, src)
            cast_flip[0] += 1

        def band_cast_small(dst, src):
            # leftover 128-wide tiles: alternate ACT/DVE (GpSimd can't
            # read PSUM on TRN2 -- compiler rejects it)
            if cast_flip[1] % 2 == 0:
                nc.scalar.copy(dst, src)
            else:
                nc.vector.tensor_copy(dst, src)
            cast_flip[1] += 1

        def emit_proj_qk(b):
            proj_tile(q2T, x_sl, "q", b)
            proj_tile(k2T, x_sl, "k", b)

        def emit_proj_v(b):
            proj_tile(v2T, x_sl, "v", b)
            for t in range(b * 4, b * 4 + 4):
                pst = ps.tile([128, 128], F16, name="ps_vt", tag="ps512",
                              bufs=3)
                nc.tensor.transpose(pst, v2T[:, t * 128:(t + 1) * 128], ident)
                nc.vector.tensor_copy(vtok[t][:, 0:64], pst[:, 0:64])
                nc.vector.tensor_copy(vtok[t][:, 65:129], pst[:, 64:128])
                nc.gpsimd.memset(vtok[t][:, 64:65], 1.0)
                nc.gpsimd.memset(vtok[t][:, 129:130], 1.0)

        # --- B1: score bands for one batch: matmul -> sbuf -> dram ---
        c2p_bd = {}         # (b, h) -> dram tile [512, 1024] (pitch-1024 skew)
        p2c_bd = {}         # (b, h) -> dram tile [128, 4*BAND]

        def emit_b1(b):
            csb_m = sb_band.tile([128, 2 * 4 * BAND], F16, name="c2p_sbm",
                                 tag="c2p_sbm", bufs=3)
            psb_m = sb_band.tile([128, 2 * 4 * BAND], F16, name="p2c_sbm",
                                 tag="p2c_sbm", bufs=3)
            psb = {h: psb_m[:, h * 4 * BAND:(h + 1) * 4 * BAND]
                   for h in range(HC)}
            for blk in range(4):
                c0 = 128 * (3 - blk)
                cs = slice(b * 512 + blk * 128, b * 512 + (blk + 1) * 128)
                for src2T, pos, stage in (
                        (q2T, posk, None), (k2T, posq, psb)):
                    for h in range(HC):
                        hs = slice(h * 64, (h + 1) * 64)
                        pm = ps.tile([128, 512], F32, name="ps_bm",
                                     tag="ps512", bufs=3)
                        nc.tensor.matmul(
                            out=pm, lhsT=src2T[hs, cs],
                            rhs=pos[hs, c0:c0 + 512], start=True, stop=True)
                        pl = ps.tile([128, 128], F32, name="ps_bl",
                                     tag="ps512", bufs=3)
                        nc.tensor.matmul(
                            out=pl, lhsT=src2T[hs, cs],
                            rhs=pos[hs, c0 + 512:c0 + BAND],
                            start=True, stop=True)
                        if stage is None:  # c2p: (h, g, j) col layout
                            o = h * 4 * BAND + blk * BAND
                            dst = csb_m
                        else:
                            o = blk * BAND
                            dst = stage[h]
                        band_cast(dst[:, o:o + 512], pm)
                        band_cast_small(dst[:, o + 512:o + BAND], pl)
            # p2c: ONE flat [128, 2*4*BAND] write per batch (both heads);
            # skew read for head h at row pitch 5120, offset h*2560
            pdr = band_dram.tile([128, 2 * 4 * BAND], F16, name=f"p2cb_{b}",
                                 tag=f"p2c_dram_{b}", bufs=1)
            nc.gpsimd.dma_start(out=pdr, in_=psb_m)
            # c2p: ONE strided write per batch covering both heads. The
            # per-head regions are spaced 523776 = 4*130944 elements apart,
            # which lets the (h, g) dims merge into a single dim of 8 at
            # stride 130944 (3-dim AP). Head h's skew read starts at
            # 523776*h + 512; writes stay disjoint (h1 writes from +384).
            cdr = band_dram.tile([2 * 524288], F16, name=f"c2pb_{b}",
                                 tag=f"c2p_dram_{b}", bufs=1)
            dst = bass.AP(cdr.tensor, cdr.offset + 384,
                          [[1024, 128], [130944, 8], [1, BAND]])
            nc.sync.dma_start(
                out=dst, in_=csb_m.rearrange("p (hg j) -> p hg j", hg=8))
            for h in range(HC):
                p2c_bd[(b, h)] = pdr
                c2p_bd[(b, h)] = (cdr, h)
                emit_reads(b, h)

        # --- B2: attention, in three emission steps per (b, h) ---
        # constant exp bias keeps f16 E and the f16-transposed Z in range;
        # it cancels exactly in the final E@v / Z normalization
        t_tiles = {}
        qk_tiles = {}
        ostages = {}

        def emit_reads(b, h, fine=False):
            # t_sb[k, kb*512 + q]: merged skew reads -- one transpose DMA
            # then one accumulating SWDGE DMA (p2c). Transposes MUST stay on
            # the SP ring: ACT-ring transpose completion signaling races
            # with any quick same-tile consumer on HW (verified 3x).
            t_sb = sb_work.tile([128, 2048], F16, name="t_sb",
                                tag="t_sb", bufs=8)
            t3 = t_sb.rearrange("p (a j) -> p a j", a=4)
            cb, ch = c2p_bd[(b, h)]
            pb = p2c_bd[(b, h)]
            nc.sync.dma_start_transpose(
                out=t3, in_=bass.AP(cb.tensor, cb.offset + ch * 523776 + 512,
                                    [[1023, 512], [1, 512]]))
            P2P = 2 * 4 * BAND  # merged p2c row pitch
            if h == 0:
                # accum in halves: TT/exp/PV for kb 0-1 start after 512KB
                # instead of the full 1MB (transpose stays whole-tile)
                for hf in range(2):
                    nc.gpsimd.dma_start(
                        out=t3[:, 2 * hf:2 * hf + 2, :],
                        in_=bass.AP(pb.tensor,
                                    pb.offset + h * 4 * BAND + 1280 * hf + 128,
                                    [[P2P - 1, 128], [640, 2], [1, 512]]),
                        accum_op=ALU.add)
                t_tiles[(b, h)] = t_sb
            else:
                # h1: plain sheared read into a separate tile -- it needs no
                # transpose ordering, so it runs before/parallel to the trs;
                # the add folds into an extra vector TT in emit_score
                t_p2c = sb_work.tile([128, 2048], F16, name="t_p2c",
                                     tag="t_p2c", bufs=4)
                nc.gpsimd.dma_start(
                    out=t_p2c.rearrange("p (a j) -> p a j", a=4),
                    in_=bass.AP(pb.tensor, pb.offset + h * 4 * BAND + 128,
                                [[P2P - 1, 128], [640, 4], [1, 512]]))
                t_tiles[(b, h)] = (t_sb, t_p2c)

        def emit_score(b, h, fine=False):
            hs = slice(h * 64, (h + 1) * 64)
            tt = t_tiles[(b, h)]
            t_sb, t_p2c = tt if h else (tt, None)
            pqk = []
            for kb in range(4):
                ks = slice(b * 512 + kb * 128, b * 512 + (kb + 1) * 128)
                ps_qk = ps.tile([128, 512], F32, name="ps_qk",
                                tag="ps_qk", bufs=3)
                nc.tensor.matmul(
                    out=ps_qk, lhsT=k2T[hs, ks],
                    rhs=q2T[hs, b * 512:(b + 1) * 512],
                    start=True, stop=True)
                pqk.append(ps_qk)
            for kb in range(4):
                sl = t_sb[:, kb * 512:(kb + 1) * 512]
                if t_p2c is not None:
                    nc.vector.tensor_tensor(
                        out=sl, in0=sl,
                        in1=t_p2c[:, kb * 512:(kb + 1) * 512], op=ALU.add)
                nc.vector.tensor_tensor(out=sl, in0=sl, in1=pqk[kb],
                                        op=ALU.add)
                nc.scalar.activation(out=sl, in_=sl, func=AF.Exp,
                                     scale=SCALE, bias=exp_bias)
            qk_tiles[(b, h)] = pqk

        def emit_pv(b, h):
            if h == 0:
                ostages[b] = sb_work.tile([128, 512], F32, name="ostage",
                                          tag="ostage", bufs=2)
            ostage = ostages[b]
            tt = t_tiles.pop((b, h))
            t_sb = tt[0] if h else tt
            qk_tiles.pop((b, h))
            ps_pv = ps.tile([65, 512], F32, name="ps_pv", tag="ps_pv", bufs=2)
            for kb in range(4):
                nc.tensor.matmul(
                    out=ps_pv, lhsT=vtok[b * 4 + kb][:, h * 65:h * 65 + 65],
                    rhs=t_sb[:, kb * 512:(kb + 1) * 512],
                    start=(kb == 0), stop=(kb == 3))

            # --- finalize: out^T [65, 512] -> transpose -> /Z -> stage ---
            o2T = sb_work.tile([65, 512], F16, name="o2T", tag="o2T", bufs=2)
            nc.vector.tensor_copy(o2T, ps_pv)
            # 66-col stride keeps every psum access 4-byte aligned
            psT = ps.tile([128, 264], F16, name="psT", tag="ps512", bufs=3)
            for qc in range(4):
                nc.tensor.transpose(psT[:, 66 * qc:66 * qc + 65],
                                    o2T[:, qc * 128:(qc + 1) * 128],
                                    ident[0:65, 0:65])
            zrec = sb_work.tile([128, 4], F32, name="zrec", tag="zrec",
                                bufs=4)
            nc.vector.reciprocal(
                zrec, bass.AP(psT.tensor, psT.offset + 64,
                              [[psT.ap[0][0], 128], [66, 4]]))
            for qc in range(4):
                nc.vector.tensor_scalar_mul(
                    ostage[:, qc * 128 + h * 64:qc * 128 + (h + 1) * 64],
                    psT[:, 66 * qc:66 * qc + 64], zrec[:, qc:qc + 1])
            if h == HC - 1:
                dst = bass.AP(out_d.tensor, out_d.offset + b * 65536,
                              [[128, 128], [16384, 4], [1, 128]])
                nc.gpsimd.dma_start(
                    out=dst, in_=ostage.rearrange("p (g j) -> p g j", g=4))

        # ---- emission: reads issue with their batch's writes; all QK/PV
        # work trails so the TM queue never parks on a young dependency ----
        for nt in range(R // 512):
            proj_tile(posk, rel_sl, "k", nt)
            proj_tile(posq, rel_sl, "q", nt)
        emit_proj_qk(0); emit_b1(0)
        emit_proj_qk(1); emit_b1(1)
        emit_proj_qk(2); emit_b1(2)
        emit_proj_qk(3); emit_b1(3)
        emit_proj_v(0); emit_proj_v(1); emit_proj_v(2); emit_proj_v(3)
        emit_score(0, 0); emit_score(0, 1)
        emit_pv(0, 0); emit_pv(0, 1)
        emit_score(1, 0); emit_score(1, 1)
        emit_pv(1, 0); emit_pv(1, 1)
        emit_score(2, 0); emit_score(2, 1)
        emit_pv(2, 0); emit_pv(2, 1)
        emit_score(3, 0); emit_score(3, 1)
        emit_pv(3, 0); emit_pv(3, 1)


_NC_CACHE = None


def _get_nc():
    global _NC_CACHE
    if _NC_CACHE is None:
        _NC_CACHE = build_nc()
    return _NC_CACHE


def make_in_maps(inputs):
    x = np.asarray(inputs["x"], np.float32)
    rel = np.asarray(inputs["rel_embeddings"], np.float32)
    Wq = np.asarray(inputs["Wq"], np.float32)
    Wk = np.asarray(inputs["Wk"], np.float32)
    Wv = np.asarray(inputs["Wv"], np.float32)
    bq = np.asarray(inputs["bq"], np.float32)
    bk = np.asarray(inputs["bk"], np.float32)
    bv = np.asarray(inputs["bv"], np.float32)

    # chunk-contiguous host layouts (see build_nc)
    xTc = (x.reshape(B, S, KC, 128).transpose(3, 0, 2, 1)
           .reshape(128, KC * T).astype(np.float16))
    relTc = (rel[::-1].T.reshape(KC, 128, R).transpose(1, 0, 2)
             .reshape(128, KC * R).astype(np.float16))

    def wchunk(Wm, sl):
        return np.ascontiguousarray(
            Wm[:, sl].reshape(KC, 128, 128).transpose(1, 0, 2)
            .reshape(128, KC * 128)).astype(np.float16)

    in_maps = []
    for c in range(NCORES):
        sl = slice(c * 128, (c + 1) * 128)
        in_maps.append({
            "xT": xTc,
            "relT": relTc,
            "Wq": wchunk(Wq, sl),
            "Wk": wchunk(Wk, sl),
            "Wv": wchunk(Wv, sl),
            "bq": np.ascontiguousarray(bq[sl]).reshape(128, 1),
            "bk": np.ascontiguousarray(bk[sl]).reshape(128, 1),
            "bv": np.ascontiguousarray(bv[sl]).reshape(128, 1),
        })
    return in_maps


def kernel(**inputs):
    nc = _get_nc()
    in_maps = make_in_maps(inputs)
    res = run_bass_kernel_spmd(nc, in_maps, list(range(NCORES))).results
    out = np.concatenate([res[c]["out"] for c in range(NCORES)], axis=1)
    return out.reshape(B, S, DIM).astype(np.float32)


# revision 49
# speedup vs baseline: 1.1375x; 1.1375x over previous
"""Disentangled MHA (DeBERTa-style) Trainium2 Bass kernel.

Sharding: 16 heads across 8 cores (2 heads/core), batch kept local.
Per core: project q/k/v with a 128-column weight slice, build the
relative-position score bands, skew-gather them via a DRAM round trip,
softmax (transposed orientation, unnormalized-exp + fused Z column),
and PV matmul. Host concatenates the per-core 128-feature outputs.

Key structure: host pre-chunks inputs so every load DMA is contiguous
per partition (x loads gated behind relT so the critical path gets HBM
bandwidth first); per-batch interleaved emission (proj -> bands ->
reads -> scores -> PV as separate pipeline steps, reads issued with
their batch's band writes and QK/PV trailing ~1.5 batches); merged
whole-(b,h) skew reads (one transpose DMA on the SP HWDGE ring + one
accumulating SWDGE DMA -- note an SWDGE accum after an ACT-ring
transpose races on HW, and GpSimd cannot touch PSUM); DMA spread over
the three issuing queues; single 8-bank PSUM pool
(proj/band 3 + qk 2 + pv 2 + transposes 1).

B=4, S=512, DIM=1024, H=16, HD=64, MAX_REL=512.
"""

import numpy as np

import concourse.bass as bass
import concourse.bacc as bacc
import concourse.mybir as mybir
import concourse.tile as tile
from concourse.bass_utils import run_bass_kernel_spmd
from concourse.masks import make_identity

B, S, DIM, H, HD = 4, 512, 1024, 16, 64
T = B * S                      # 2048 tokens
R = 1024                       # 2 * att_span rel rows
HC = 2                         # heads per core
NCORES = 8
KC = DIM // 128                # contraction chunks
SCALE = float((HD * 3) ** (-0.5))
BAND = 640                     # skew band width (needs >= 512 + 127)

F32 = mybir.dt.float32
F16 = mybir.dt.float16
AF = mybir.ActivationFunctionType
ALU = mybir.AluOpType


def build_nc():
    nc = bacc.Bacc("TRN2", target_bir_lowering=False, debug=False)

    # host pre-chunked layouts: row p holds chunk-contiguous data
    xT_d = nc.dram_tensor("xT", [128, KC * T], F16, kind="ExternalInput")
    relT_d = nc.dram_tensor("relT", [128, KC * R], F16, kind="ExternalInput")
    W_d = {
        n: nc.dram_tensor(f"W{n}", [128, KC * 128], F16, kind="ExternalInput")
        for n in "qkv"
    }
    b_d = {
        n: nc.dram_tensor(f"b{n}", [128, 1], F32, kind="ExternalInput")
        for n in "qkv"
    }
    out_d = nc.dram_tensor("out", [T, 128], F32, kind="ExternalOutput")

    with tile.TileContext(nc) as tc:
        _body(nc, tc, xT_d.ap(), relT_d.ap(),
              {n: W_d[n].ap() for n in "qkv"},
              {n: b_d[n].ap() for n in "qkv"},
              out_d.ap())
    nc.compile()
    return nc


def _body(nc, tc, xT, relT, W, bvec, out_d):
    from contextlib import ExitStack
    ctx = ExitStack()
    with ctx:
        singles = ctx.enter_context(tc.tile_pool(name="singles", bufs=1))

        # ---- Input loads: all contiguous, spread over the three queues.
        relT_sb = singles.tile([128, KC * R], F16, name="relT_sb")
        W_sb = {n: singles.tile([128, KC * 128], F16, name=f"W{n}_sb")
                for n in "qkv"}
        xT_sb = singles.tile([128, KC * T], F16, name="xT_sb")
        b_t = {n: singles.tile([128, 1], F32, name=f"b{n}") for n in "qkv"}

        HKR = KC * R // 2
        QKR = KC * R // 4
        nc.sync.dma_start(out=W_sb["k"], in_=W["k"])
        nc.scalar.dma_start(out=W_sb["q"], in_=W["q"])
        # relT in quarters, alternating rings: chunk 0 lands first so the
        # posk projection chain starts streaming via subtile deps
        nc.sync.dma_start(out=relT_sb[:, 0:QKR], in_=relT[:, 0:QKR])
        nc.scalar.dma_start(out=relT_sb[:, QKR:HKR], in_=relT[:, QKR:HKR])
        nc.sync.dma_start(out=relT_sb[:, HKR:3 * QKR],
                          in_=relT[:, HKR:3 * QKR])
        nc.scalar.dma_start(out=relT_sb[:, 3 * QKR:], in_=relT[:, 3 * QKR:])
        nc.gpsimd.dma_start(out=W_sb["v"], in_=W["v"])
        BK = KC * S  # per-batch chunk-contiguous span
        # all xT loads gated behind relT: tiny copies reading relT_sb force
        # the x SDMA transfers to start only after the critical relT load
        # drains, so relT/W don't fair-share HBM bandwidth with 4MB of x.
        # xb0 releases at half-relT, xb1-3 at the tail (subtile deps).
        nc.vector.tensor_copy(xT_sb[0:1, 0:1], relT_sb[0:1, HKR - 1:HKR])
        for b in range(1, B):
            nc.vector.tensor_copy(xT_sb[0:1, b * BK:b * BK + 1],
                                  relT_sb[0:1, KC * R - 1:KC * R])
        xq = [nc.sync, nc.scalar, nc.gpsimd, nc.gpsimd]
        for b in range(B):
            xq[b].dma_start(out=xT_sb[:, b * BK:(b + 1) * BK],
                            in_=xT[:, b * BK:(b + 1) * BK])
        for n in "qkv":
            nc.gpsimd.dma_start(out=b_t[n], in_=bvec[n])

        def x_sl(kc, b):
            return xT_sb[:, (b * KC + kc) * S:(b * KC + kc) * S + S]

        def rel_sl(kc, nt):
            return relT_sb[:, kc * R + nt * 512:kc * R + nt * 512 + 512]

        W_t = {n: [W_sb[n][:, i * 128:(i + 1) * 128] for i in range(KC)]
               for n in "qkv"}

        ident = singles.tile([128, 128], F16, name="ident")
        make_identity(nc, ident)

        exp_bias = singles.tile([128, 1], F32, name="exp_bias")
        nc.gpsimd.memset(exp_bias, -4.0)

        # ---- persistent SBUF for projections ----
        q2T = singles.tile([128, T], F16, name="q2T")
        k2T = singles.tile([128, T], F16, name="k2T")
        v2T = singles.tile([128, T], F16, name="v2T")
        posk = singles.tile([128, R], F16, name="posk")
        posq = singles.tile([128, R], F16, name="posq")
        vtok = [singles.tile([128, 130], F16, name=f"vtok{t}")
                for t in range(T // 128)]

        # ---- single PSUM pool: ps512(3, shared with f16 transposes) +
        # ps_qk(3) + ps_pv(2) = 8 banks
        ps = ctx.enter_context(tc.tile_pool(name="ps", space="PSUM", bufs=1))

        sb_band = ctx.enter_context(tc.tile_pool(name="sb_band", bufs=1))
        sb_work = ctx.enter_context(tc.tile_pool(name="sb_work", bufs=1))
        band_dram = ctx.enter_context(
            tc.tile_pool(name="bands", space="DRAM", bufs=1))

        # ---- helpers ----
        def proj_tile(out_sb, rhs_sl, wn, nt):
            psx = ps.tile([128, 512], F32, name="ps_proj", tag="ps512", bufs=3)
            for kc in range(KC):
                nc.tensor.matmul(
                    out=psx, lhsT=W_t[wn][kc], rhs=rhs_sl(kc, nt),
                    start=(kc == 0), stop=(kc == KC - 1),
                )
            nc.scalar.activation(
                out=out_sb[:, nt * 512:(nt + 1) * 512], in_=psx,
                func=AF.Identity, bias=b_t[wn], scale=1.0,
            )

        cast_flip = [0, 0]

        def band_cast(dst, src):
            # 2:1 DVE:ACT for the 512-wide band psum evacuation
            if cast_flip[0] % 3 != 2:
                nc.vector.tensor_copy(dst, src)
            else:
                nc.scalar.copy(dst, src)
            cast_flip[0] += 1

        def band_cast_small(dst, src):
            # leftover 128-wide tiles: alternate ACT/DVE (GpSimd can't
            # read PSUM on TRN2 -- compiler rejects it)
            if cast_flip[1] % 2 == 0:
                nc.scalar.copy(dst, src)
            else:
                nc.vector.tensor_copy(dst, src)
            cast_flip[1] += 1

        def emit_proj_qk(b):
            proj_tile(q2T, x_sl, "q", b)
            proj_tile(k2T, x_sl, "k", b)

        def emit_proj_v(b):
            proj_tile(v2T, x_sl, "v", b)
            for t in range(b * 4, b * 4 + 4):
                pst = ps.tile([128, 128], F16, name="ps_vt", tag="ps512",
                              bufs=3)
                nc.tensor.transpose(pst, v2T[:, t * 128:(t + 1) * 128], ident)
                nc.vector.tensor_copy(vtok[t][:, 0:64], pst[:, 0:64])
                nc.vector.tensor_copy(vtok[t][:, 65:129], pst[:, 64:128])
                nc.gpsimd.memset(vtok[t][:, 64:65], 1.0)
                nc.gpsimd.memset(vtok[t][:, 129:130], 1.0)

        # --- B1: score bands for one batch: matmul -> sbuf -> dram ---
        c2p_bd = {}         # (b, h) -> dram tile [512, 1024] (pitch-1024 skew)
        p2c_bd = {}         # (b, h) -> dram tile [128, 4*BAND]

        def emit_b1(b):
            csb_m = sb_band.tile([128, 2 * 4 * BAND], F16, name="c2p_sbm",
                                 tag="c2p_sbm", bufs=3)
            psb_m = sb_band.tile([128, 2 * 4 * BAND], F16, name="p2c_sbm",
                                 tag="p2c_sbm", bufs=3)
            psb = {h: psb_m[:, h * 4 * BAND:(h + 1) * 4 * BAND]
                   for h in range(HC)}
            for blk in range(4):
                c0 = 128 * (3 - blk)
                cs = slice(b * 512 + blk * 128, b * 512 + (blk + 1) * 128)
                for src2T, pos, stage in (
                        (q2T, posk, None), (k2T, posq, psb)):
                    for h in range(HC):
                        hs = slice(h * 64, (h + 1) * 64)
                        pm = ps.tile([128, 512], F32, name="ps_bm",
                                     tag="ps512", bufs=3)
                        nc.tensor.matmul(
                            out=pm, lhsT=src2T[hs, cs],
                            rhs=pos[hs, c0:c0 + 512], start=True, stop=True)
                        pl = ps.tile([128, 128], F32, name="ps_bl",
                                     tag="ps512", bufs=3)
                        nc.tensor.matmul(
                            out=pl, lhsT=src2T[hs, cs],
                            rhs=pos[hs, c0 + 512:c0 + BAND],
                            start=True, stop=True)
                        if stage is None:  # c2p: (h, g, j) col layout
                            o = h * 4 * BAND + blk * BAND
                            dst = csb_m
                        else:
                            o = blk * BAND
                            dst = stage[h]
                        band_cast(dst[:, o:o + 512], pm)
                        band_cast_small(dst[:, o + 512:o + BAND], pl)
            # p2c: ONE flat [128, 2*4*BAND] write per batch (both heads);
            # skew read for head h at row pitch 5120, offset h*2560
            pdr = band_dram.tile([128, 2 * 4 * BAND], F16, name=f"p2cb_{b}",
                                 tag=f"p2c_dram_{b}", bufs=1)
            nc.gpsimd.dma_start(out=pdr, in_=psb_m)
            # c2p: ONE strided write per batch covering both heads. The
            # per-head regions are spaced 523776 = 4*130944 elements apart,
            # which lets the (h, g) dims merge into a single dim of 8 at
            # stride 130944 (3-dim AP). Head h's skew read starts at
            # 523776*h + 512; writes stay disjoint (h1 writes from +384).
            cdr = band_dram.tile([2 * 524288], F16, name=f"c2pb_{b}",
                                 tag=f"c2p_dram_{b}", bufs=1)
            dst = bass.AP(cdr.tensor, cdr.offset + 384,
                          [[1024, 128], [130944, 8], [1, BAND]])
            nc.sync.dma_start(
                out=dst, in_=csb_m.rearrange("p (hg j) -> p hg j", hg=8))
            for h in range(HC):
                p2c_bd[(b, h)] = pdr
                c2p_bd[(b, h)] = (cdr, h)
                emit_reads(b, h)

        # --- B2: attention, in three emission steps per (b, h) ---
        # constant exp bias keeps f16 E and the f16-transposed Z in range;
        # it cancels exactly in the final E@v / Z normalization
        t_tiles = {}
        qk_tiles = {}
        ostages = {}

        def emit_reads(b, h, fine=False):
            # t_sb[k, kb*512 + q]: merged skew reads -- one transpose DMA
            # then one accumulating SWDGE DMA (p2c). Transposes MUST stay on
            # the SP ring: ACT-ring transpose completion signaling races
            # with any quick same-tile consumer on HW (verified 3x).
            t_sb = sb_work.tile([128, 2048], F16, name="t_sb",
                                tag="t_sb", bufs=8)
            t3 = t_sb.rearrange("p (a j) -> p a j", a=4)
            cb, ch = c2p_bd[(b, h)]
            pb = p2c_bd[(b, h)]
            nc.sync.dma_start_transpose(
                out=t3, in_=bass.AP(cb.tensor, cb.offset + ch * 523776 + 512,
                                    [[1023, 512], [1, 512]]))
            # accum in halves: TT/exp/PV for kb 0-1 start after 512KB
            # instead of the full 1MB (transpose stays whole-tile)
            P2P = 2 * 4 * BAND  # merged p2c row pitch
            for hf in range(2):
                nc.gpsimd.dma_start(
                    out=t3[:, 2 * hf:2 * hf + 2, :],
                    in_=bass.AP(pb.tensor,
                                pb.offset + h * 4 * BAND + 1280 * hf + 128,
                                [[P2P - 1, 128], [640, 2], [1, 512]]),
                    accum_op=ALU.add)
            t_tiles[(b, h)] = t_sb

        def emit_score(b, h, fine=False):
            hs = slice(h * 64, (h + 1) * 64)
            t_sb = t_tiles[(b, h)]
            pqk = []
            for kb in range(4):
                ks = slice(b * 512 + kb * 128, b * 512 + (kb + 1) * 128)
                ps_qk = ps.tile([128, 512], F32, name="ps_qk",
                                tag="ps_qk", bufs=3)
                nc.tensor.matmul(
                    out=ps_qk, lhsT=k2T[hs, ks],
                    rhs=q2T[hs, b * 512:(b + 1) * 512],
                    start=True, stop=True)
                pqk.append(ps_qk)
            for kb in range(4):
                sl = t_sb[:, kb * 512:(kb + 1) * 512]
                nc.vector.tensor_tensor(out=sl, in0=sl, in1=pqk[kb],
                                        op=ALU.add)
                nc.scalar.activation(out=sl, in_=sl, func=AF.Exp,
                                     scale=SCALE, bias=exp_bias)
            qk_tiles[(b, h)] = pqk

        def emit_pv(b, h):
            if h == 0:
                ostages[b] = sb_work.tile([128, 512], F32, name="ostage",
                                          tag="ostage", bufs=2)
            ostage = ostages[b]
            t_sb = t_tiles.pop((b, h))
            qk_tiles.pop((b, h))
            ps_pv = ps.tile([65, 512], F32, name="ps_pv", tag="ps_pv", bufs=2)
            for kb in range(4):
                nc.tensor.matmul(
                    out=ps_pv, lhsT=vtok[b * 4 + kb][:, h * 65:h * 65 + 65],
                    rhs=t_sb[:, kb * 512:(kb + 1) * 512],
                    start=(kb == 0), stop=(kb == 3))

            # --- finalize: out^T [65, 512] -> transpose -> /Z -> stage ---
            o2T = sb_work.tile([65, 512], F16, name="o2T", tag="o2T", bufs=2)
            nc.vector.tensor_copy(o2T, ps_pv)
            # 66-col stride keeps every psum access 4-byte aligned
            psT = ps.tile([128, 264], F16, name="psT", tag="ps512", bufs=3)
            for qc in range(4):
                nc.tensor.transpose(psT[:, 66 * qc:66 * qc + 65],
                                    o2T[:, qc * 128:(qc + 1) * 128],
                                    ident[0:65, 0:65])
            zrec = sb_work.tile([128, 4], F32, name="zrec", tag="zrec",
                                bufs=4)
            nc.vector.reciprocal(
                zrec, bass.AP(psT.tensor, psT.offset + 64,
                              [[psT.ap[0][0], 128], [66, 4]]))
            for qc in range(4):
                nc.vector.tensor_scalar_mul(
                    ostage[:, qc * 128 + h * 64:qc * 128 + (h + 1) * 64],
                    psT[:, 66 * qc:66 * qc + 64], zrec[:, qc:qc + 1])
            if h == HC - 1:
                dst = bass.AP(out_d.tensor, out_d.offset + b * 65536,
                              [[128, 128], [16384, 4], [1, 128]])
                nc.gpsimd.dma_start(
                    out=dst, in_=ostage.rearrange("p (g j) -> p g j", g=4))

        # ---- emission: reads issue with their batch's writes; all QK/PV
        # work trails so the TM queue never parks on a young dependency ----
        for nt in range(R // 512):
            proj_tile(posk, rel_sl, "k", nt)
            proj_tile(posq, rel_sl, "q", nt)
        emit_proj_qk(0); emit_b1(0)
        emit_proj_qk(1); emit_b1(1)
        emit_proj_qk(2); emit_b1(2)
        emit_proj_qk(3); emit_b1(3)
        emit_proj_v(0); emit_proj_v(1); emit_proj_v(2); emit_proj_v(3)
        emit_score(0, 0); emit_score(0, 1)
        emit_pv(0, 0); emit_pv(0, 1)
        emit_score(1, 0); emit_score(1, 1)
        emit_pv(1, 0); emit_pv(1, 1)
        emit_score(2, 0); emit_score(2, 1)
        emit_pv(2, 0); emit_pv(2, 1)
        emit_score(3, 0); emit_score(3, 1)
        emit_pv(3, 0); emit_pv(3, 1)


_NC_CACHE = None


def _get_nc():
    global _NC_CACHE
    if _NC_CACHE is None:
        _NC_CACHE = build_nc()
    return _NC_CACHE


def make_in_maps(inputs):
    x = np.asarray(inputs["x"], np.float32)
    rel = np.asarray(inputs["rel_embeddings"], np.float32)
    Wq = np.asarray(inputs["Wq"], np.float32)
    Wk = np.asarray(inputs["Wk"], np.float32)
    Wv = np.asarray(inputs["Wv"], np.float32)
    bq = np.asarray(inputs["bq"], np.float32)
    bk = np.asarray(inputs["bk"], np.float32)
    bv = np.asarray(inputs["bv"], np.float32)

    # chunk-contiguous host layouts (see build_nc)
    xTc = (x.reshape(B, S, KC, 128).transpose(3, 0, 2, 1)
           .reshape(128, KC * T).astype(np.float16))
    relTc = (rel[::-1].T.reshape(KC, 128, R).transpose(1, 0, 2)
             .reshape(128, KC * R).astype(np.float16))

    def wchunk(Wm, sl):
        return np.ascontiguousarray(
            Wm[:, sl].reshape(KC, 128, 128).transpose(1, 0, 2)
            .reshape(128, KC * 128)).astype(np.float16)

    in_maps = []
    for c in range(NCORES):
        sl = slice(c * 128, (c + 1) * 128)
        in_maps.append({
            "xT": xTc,
            "relT": relTc,
            "Wq": wchunk(Wq, sl),
            "Wk": wchunk(Wk, sl),
            "Wv": wchunk(Wv, sl),
            "bq": np.ascontiguousarray(bq[sl]).reshape(128, 1),
            "bk": np.ascontiguousarray(bk[sl]).reshape(128, 1),
            "bv": np.ascontiguousarray(bv[sl]).reshape(128, 1),
        })
    return in_maps


def kernel(**inputs):
    nc = _get_nc()
    in_maps = make_in_maps(inputs)
    res = run_bass_kernel_spmd(nc, in_maps, list(range(NCORES))).results
    out = np.concatenate([res[c]["out"] for c in range(NCORES)], axis=1)
    return out.reshape(B, S, DIM).astype(np.float32)
